# revision 1
# baseline (speedup 1.0000x reference)
"""Trainium2 Bass kernel: 3D Gaussian mixture rendered on a voxel grid.

Computes grid[z,y,x] = sum_a amp * prod_axis (voxel-averaged 1D gaussian
integrals via erf), i.e. a sum of 2048 separable outer products.

Strategy:
  - Shard the output grid along y: core i renders y-pixels [16i, 16i+16).
    No collectives; host concatenates the 8 disjoint slabs.
  - Host-side atom culling per slab: only atoms within MARGIN_SIGMA*sigma
    of the slab matter; each core keeps the 512 closest atoms (atoms
    beyond ~5 sigma contribute < 1e-6 relatively) -> NBLK=4 blocks of 128.
    Pad atoms get y=1e4, so their y erf-diff saturates to exactly 0.
  - Device pipeline, per 128-atom block (blocks pipeline across engines):
      ACT:  erf at pixel *edges* (one erf per edge; the difference of
            adjacent edge evals gives the voxel-averaged integral). x, z
            and y edge evals land in one combined tile per block.
      DVE:  one shifted-slice subtraction produces gx|gz|gy (fp16 out);
            broadcast-AP tensor_tensor ops build the Khatri-Rao
            H[y,x] = gx[x] * gy[y] (y0-5, y8-15), while the otherwise-idle
            ScalarE builds y6-7 via Copy-with-per-partition-scale.
      PE:   grid[z, (y,x)] += gz_b.T @ H_b accumulated in 4 PSUM banks
            over blocks (contraction over atoms), fp16 at full PE rate.
      PSUM -> SBUF copies (DVE+ACT, applying the global amp*(0.5/vs)^3
      scale for free) -> HBM on three parallel DMA queues.
"""

import os

import numpy as np

import concourse.bacc as bacc
import concourse.bass as bass
import concourse.tile as tile
from concourse import mybir
from concourse.bass_utils import run_bass_kernel_spmd

N_PIX = 128
N_CORES = 8
SLAB = N_PIX // N_CORES  # 16 y-pixels per core
NBLK = 4  # atom blocks of 128 per core
CAP = NBLK * 128
MARGIN_SIGMA = 6.5  # cull atoms farther than this (in sigmas) from the slab

LAST_RESULTS = None  # BassKernelResults of the most recent run (for test.py)

# merged-input column layout: small control part (pos/mask/yedges) first so
# its DMA lands before the edges part
_C_PX = 0
_C_PY = _C_PX + NBLK
_C_PZ = _C_PY + NBLK
_C_MASK = _C_PZ + NBLK
_C_YEDGE = _C_MASK + NBLK
_C_EDGE = _C_YEDGE + SLAB + 1
_W_CTL = _C_EDGE
_W_IN = _C_EDGE + N_PIX + 1

# combined x|y|z edge-eval tile layout: x erf at [0:129], y erf at
# [132:149], z erf at [152:281]. y sits before z so the x+y diff (all H
# needs) never waits for the z erf.
_YOFF = 132
_ZOFF = 152
_XZ_W = 284


def _bcast_mid(ap: bass.AP, n: int) -> bass.AP:
    """[128, F] AP -> [128, n, F] with a step-0 middle dim."""
    return bass.AP(
        tensor=ap.tensor, offset=ap.offset, ap=[ap.ap[0], [0, n], *ap.ap[1:]]
    )


def _build_nc(inv_d: float, c_amp: float):
    f32 = mybir.dt.float32
    f16 = mybir.dt.float16
    Erf = mybir.ActivationFunctionType.Erf
    mult = mybir.AluOpType.mult

    nc = bacc.Bacc(None, target_bir_lowering=False, name="gauss3d")
    inp_d = nc.dram_tensor("inp", [128, _W_IN], f32, kind="ExternalInput")
    grid_d = nc.dram_tensor("grid", [128, SLAB * N_PIX], f32, kind="ExternalOutput")

    with tile.TileContext(nc) as tc:
        with (
            tc.tile_pool(name="const", bufs=1) as const,
            tc.tile_pool(name="work", bufs=3) as work,
            tc.tile_pool(name="o", bufs=2) as opool,
            tc.tile_pool(name="ps", bufs=1, space="PSUM") as psum,
        ):
            # dependency-free erf so both ACT tables load during the input
            # DMA flight instead of stalling the first real erf
            warm = const.tile([128, 1], f32)
            nc.scalar.activation(
                warm[:], nc.const_aps.scalar_like(0.0, warm[:]), Erf
            )

            inp = const.tile([128, _W_IN], f32)
            nc.scalar.dma_start(inp[:, 0:_W_CTL], inp_d[:, 0:_W_CTL])
            nc.sync.dma_start(inp[:, _W_CTL:_W_IN], inp_d[:, _W_CTL:_W_IN])
            edges = inp[:, _C_EDGE : _C_EDGE + N_PIX + 1]
            yedges = inp[:, _C_YEDGE : _C_YEDGE + SLAB + 1]
            posx = inp[:, _C_PX : _C_PX + NBLK]
            posy = inp[:, _C_PY : _C_PY + NBLK]
            posz = inp[:, _C_PZ : _C_PZ + NBLK]

            # activation computes func(in*scale + bias): bias_col = -pos*inv_d.
            # pos x|y|z are contiguous columns -> one op for all three biases.
            bias = const.tile([128, 3 * NBLK], f32)
            nc.vector.tensor_scalar_mul(bias[:], inp[:, _C_PX : _C_PX + 3 * NBLK], -inv_d)
            bx = bias[:, 0:NBLK]
            by = bias[:, NBLK : 2 * NBLK]
            bz = bias[:, 2 * NBLK : 3 * NBLK]

            pss = [
                psum.tile([128, 512], f32, tag=f"ps{c}", name=f"ps{c}")
                for c in range(4)
            ]

            # ---- shared per-block edge evals + diffs (gxz alive all phases)
            # Phase A1 (y0-7 -> banks 0-1) runs per block here; banks 2 and 3
            # follow as separate phases so each bank's 256KB ships as soon as
            # it is final, keeping both HWDGE queues continuously fed.
            gxzs = []
            hs = []
            for b in range(NBLK):
                exz = work.tile([128, _XZ_W], f32, tag="exz")
                nc.scalar.activation(
                    exz[:, 0 : N_PIX + 1], edges, Erf, bias=bx[:, b : b + 1], scale=inv_d
                )
                nc.scalar.activation(
                    exz[:, _YOFF : _YOFF + SLAB + 1],
                    yedges,
                    Erf,
                    bias=by[:, b : b + 1],
                    scale=inv_d,
                )
                nc.scalar.activation(
                    exz[:, _ZOFF : _ZOFF + N_PIX + 1],
                    edges,
                    Erf,
                    bias=bz[:, b : b + 1],
                    scale=inv_d,
                )

                # diff[i] = E[i+1]-E[i]: gx = gxz[0:128], gy = gxz[132:148],
                # gz = gxz[152:280] (pads have y=1e4 -> saturated erf -> gy=0)
                gxz = work.tile([128, _XZ_W], f16, tag=f"gxz{b}", name=f"gxz{b}", bufs=1)
                if b == 0:
                    # split so block 0's H (needs x+y only) starts before the
                    # z erf finishes -- faster pipeline fill
                    nc.vector.tensor_sub(
                        gxz[:, 0 : _YOFF + SLAB],
                        exz[:, 1 : _YOFF + SLAB + 1],
                        exz[:, 0 : _YOFF + SLAB],
                    )
                    nc.vector.tensor_sub(
                        gxz[:, _ZOFF : _ZOFF + N_PIX],
                        exz[:, _ZOFF + 1 : _ZOFF + N_PIX + 1],
                        exz[:, _ZOFF : _ZOFF + N_PIX],
                    )
                else:
                    # steady state: one op for all three axes (junk in the
                    # [148:152] gap cols is never read)
                    nc.vector.tensor_sub(
                        gxz[:, 0 : _ZOFF + N_PIX],
                        exz[:, 1 : _ZOFF + N_PIX + 1],
                        exz[:, 0 : _ZOFF + N_PIX],
                    )
                gxzs.append(gxz)
                hs.append(
                    work.tile(
                        [128, SLAB, N_PIX], f16, tag=f"h{b}", name=f"h{b}", bufs=1
                    )
                )

                # phase A1: y0-7 on DVE -> banks 0-1
                h = hs[b]
                nc.vector.tensor_tensor(
                    h[:, 0:8, :],
                    _bcast_mid(gxz[:, 0:N_PIX], 8),
                    gxz[:, _YOFF : _YOFF + 8].broadcast_to([128, 8, N_PIX]),
                    mult,
                )
                for c in (0, 1):
                    nc.tensor.matmul(
                        pss[c][:],
                        lhsT=gxz[:, _ZOFF : _ZOFF + N_PIX],
                        rhs=h[:, 4 * c : 4 * c + 4, :],
                        start=(b == 0),
                        stop=(b == NBLK - 1),
                        skip_group_check=True,
                    )

            c1_dma = None
            for c in (0, 1):
                ot = opool.tile([128, 512], f32, tag=f"ot{c}", name=f"ot{c}")
                nc.scalar.mul(ot[:], pss[c][:], c_amp)
                dma = (nc.sync if c == 0 else nc.scalar).dma_start(
                    grid_d[:, 512 * c : 512 * (c + 1)], ot[:]
                )
                if c == 1:
                    c1_dma = dma

            # ---- phase A2: y8-11 -> bank 2 (DVE y8-9, ScalarE y10-11)
            for b in range(NBLK):
                gxz = gxzs[b]
                h = hs[b]
                nc.vector.tensor_tensor(
                    h[:, 8:10, :],
                    _bcast_mid(gxz[:, 0:N_PIX], 2),
                    gxz[:, _YOFF + 8 : _YOFF + 10].broadcast_to([128, 2, N_PIX]),
                    mult,
                )
                gyf = work.tile([128, 2], f32, tag="gyf")
                nc.scalar.copy(gyf[:], gxz[:, _YOFF + 10 : _YOFF + 12])
                for y in (10, 11):
                    nc.scalar.mul(h[:, y, :], gxz[:, 0:N_PIX], gyf[:, y - 10 : y - 9])
                nc.tensor.matmul(
                    pss[2][:],
                    lhsT=gxz[:, _ZOFF : _ZOFF + N_PIX],
                    rhs=h[:, 8:12, :],
                    start=(b == 0),
                    stop=(b == NBLK - 1),
                    skip_group_check=True,
                )
            # copies on ScalarE only -- a DVE copy here head-of-line-blocks
            # phase B's H ops behind its PSUM dependency. Order the copies
            # after c1's DMA issue so they don't block the scalar queue.
            ot2 = opool.tile([128, 512], f32, tag="ot2", name="ot2")
            for half in range(2):
                sl = slice(256 * half, 256 * half + 256)
                cp = nc.scalar.mul(ot2[:, sl], pss[2][:, sl], c_amp)
                if c1_dma is not None:
                    tile.add_dep_helper(
                        cp.ins,
                        c1_dma.ins,
                        sync=False,
                        reason="c2 copy after c1 dma issue (queue order)",
                    )
                (nc.sync if half == 0 else nc.scalar).dma_start(
                    grid_d[:, 1024 + 256 * half : 1024 + 256 * half + 256], ot2[:, sl]
                )

            # ---- phase B: y12-15 -> bank 3 (erf/diffs already done)
            for b in range(NBLK):
                gxz = gxzs[b]
                h = hs[b]
                nc.vector.tensor_tensor(
                    h[:, 12:16, :],
                    _bcast_mid(gxz[:, 0:N_PIX], 4),
                    gxz[:, _YOFF + 12 : _YOFF + 16].broadcast_to([128, 4, N_PIX]),
                    mult,
                )
                nc.tensor.matmul(
                    pss[3][:],
                    lhsT=gxz[:, _ZOFF : _ZOFF + N_PIX],
                    rhs=h[:, 12:16, :],
                    start=(b == 0),
                    stop=(b == NBLK - 1),
                    skip_group_check=True,
                )

            # ---- phase B flush: only 256KB left; halves on both queues
            ot3 = opool.tile([128, 512], f32, tag="ot3", name="ot3")
            for half in range(2):
                sl = slice(256 * half, 256 * half + 256)
                if half == 0:
                    nc.vector.tensor_scalar_mul(ot3[:, sl], pss[3][:, sl], c_amp)
                    nc.sync.dma_start(grid_d[:, 1536 : 1536 + 256], ot3[:, sl])
                else:
                    nc.scalar.mul(ot3[:, sl], pss[3][:, sl], c_amp)
                    nc.scalar.dma_start(grid_d[:, 1792 : 1792 + 256], ot3[:, sl])

    nc.compile()
    return nc


def _shard_inputs(pos: np.ndarray, sigma: float, vs: float, n_pix: int, c_amp: float):
    """Per-core [128, _W_IN] merged input: edge tiles + culled/padded atoms."""
    edges = ((np.arange(n_pix + 1, dtype=np.float32) - n_pix // 2) - 0.5) * np.float32(vs)

    w = np.float32(MARGIN_SIGMA * sigma)
    in_maps = []
    for i in range(N_CORES):
        e_lo = edges[SLAB * i]
        e_hi = edges[SLAB * i + SLAB]
        py = pos[:, 1]
        m = (py >= e_lo - w) & (py <= e_hi + w)
        idx = np.nonzero(m)[0]
        if len(idx) > CAP:
            # keep the CAP atoms closest to the slab; dropped atoms sit
            # beyond ~5 sigma and contribute < 1e-6 relatively
            d = np.maximum(0.0, np.maximum(e_lo - py[idx], py[idx] - e_hi))
            idx = idx[np.argsort(d, kind="stable")[:CAP]]
        n = len(idx)
        p = np.zeros((CAP, 3), dtype=np.float32)
        p[:n] = pos[idx]
        # pads: y far outside the grid -> saturated erf -> gy == 0 exactly
        p[n:, 1] = np.float32(1.0e4)
        mask = np.zeros((CAP,), dtype=np.float32)
        mask[:n] = np.float32(c_amp)

        def blk(v):  # [CAP] -> [128, NBLK] (partition = index within block)
            return v.reshape(NBLK, 128).T

        buf = np.zeros((128, _W_IN), dtype=np.float32)
        buf[:, _C_EDGE : _C_EDGE + n_pix + 1] = edges[None, :]
        buf[:, _C_YEDGE : _C_YEDGE + SLAB + 1] = edges[None, SLAB * i : SLAB * i + SLAB + 1]
        buf[:, _C_PX : _C_PX + NBLK] = blk(p[:, 0])
        buf[:, _C_PY : _C_PY + NBLK] = blk(p[:, 1])
        buf[:, _C_PZ : _C_PZ + NBLK] = blk(p[:, 2])
        buf[:, _C_MASK : _C_MASK + NBLK] = blk(mask)
        in_maps.append({"inp": buf})
    return in_maps


def kernel(
    atom_positions: np.ndarray,
    log_var: np.ndarray,
    log_weight: np.ndarray,
    n_pix,
    voxel_size,
) -> np.ndarray:
    global LAST_RESULTS
    pos = np.asarray(atom_positions, dtype=np.float32)
    lv = float(np.asarray(log_var, dtype=np.float32).reshape(-1)[0])
    lw = float(np.asarray(log_weight, dtype=np.float32).reshape(-1)[0])
    n_pix = int(n_pix)
    vs = float(voxel_size)
    assert n_pix == N_PIX, f"kernel compiled for n_pix={N_PIX}, got {n_pix}"

    sigma = float(np.exp(0.5 * lv))
    amp = float(np.exp(lw))
    inv_d = float(1.0 / (np.sqrt(2.0) * sigma))
    c_amp = float(amp * (0.5 / vs) ** 3)

    in_maps = _shard_inputs(pos, sigma, vs, n_pix, c_amp)
    nc = _build_nc(inv_d, c_amp)
    res = run_bass_kernel_spmd(
        nc,
        in_maps,
        core_ids=list(range(N_CORES)),
        trace=bool(int(os.environ.get("GAUSS3D_TRACE", "0"))),
    )
    LAST_RESULTS = res
    grids = [r["grid"].reshape(N_PIX, SLAB, N_PIX) for r in res.results]
    return np.ascontiguousarray(np.concatenate(grids, axis=1), dtype=np.float32)

